# revision 13
# baseline (speedup 1.0000x reference)
"""Distributed Trainium2 kernel for nn_AttentionBlock (B=2, N=2048, D=1024, H=16).

Sharding: 1 batch x 4 heads per core (batch = core//4, head group = core%4).
Each core computes QKV for its 4 heads over its batch's 2048 tokens, full
attention for those 4 (b, h) units, and a partial out-projection contracting
its 256 local attention-out dims.  The 4 partial [2048, 1024] products per
batch are summed on the host (the tensor-parallel all-reduce) along with b_o.
All qkv biases are applied exactly on device (q/k/v are drained e-major).

v3: the PE runs *only* matmuls (QKV / scores / PV / out-proj).  Every
transpose goes through the DMA XBAR (dma_start_transpose, 16x128 tiles,
bf16): V (e-major -> k-major with a ones column for row sums), the PV
output (hd-major -> q-major for the 1/sumexp normalize), and the attention
output (q-major -> e-major for the out-proj stationary).  exp runs on ACT
over two-bank [128,1024] PSUM tiles.  PSUM: scores 2x2 + PV 2 + misc 2 = 8
banks.  Out-proj + y DMA are interleaved per 128-row block to keep the tail
shallow.

Per-core DRAM layouts:
  xt    [4, 128, 8, 512]  x[b]^T chunked by n-block (d_inner, d_outer, n) bf16
  wqkv  [128, 8, 768]     W_qkv rows for 4 heads, (d_inner, d_outer, e)   bf16
                          e-order: q0 q1 k0 k1 v0 v1 (128 each)
  bqkv  [128, 6]          bias per e-tile (q-parts pre-scaled by 1/8)     f32
  wo    [128, 2, 1024]    W_o[:, local cols]^T (e_inner, e_outer, d_out)  bf16
  y     [2048, 1024]      partial out-proj                                bf16
"""

import numpy as np
import ml_dtypes

import concourse.bass as bass
import concourse.tile as tile
from concourse import bacc, mybir
from concourse.bass_utils import run_bass_kernel_spmd

B, N, D = 2, 2048, 1024
H, HD = 16, 64
NCORES = 8
HPC = 4  # heads per core

F32 = mybir.dt.float32
BF16 = mybir.dt.bfloat16
AF = mybir.ActivationFunctionType
MUL = mybir.AluOpType.mult
ADD = mybir.AluOpType.add


def build_nc():
    nc = bacc.Bacc(
        "TRN2", target_bir_lowering=False, debug=False, num_devices=NCORES
    )
    xt = nc.dram_tensor("xt", [4, 128, 8, 512], BF16, kind="ExternalInput").ap()
    wqkv = nc.dram_tensor("wqkv", [128, 8, 768], BF16, kind="ExternalInput").ap()
    bqkv = nc.dram_tensor("bqkv", [128, 6], F32, kind="ExternalInput").ap()
    wo = nc.dram_tensor("wo", [128, 2, 1024], BF16, kind="ExternalInput").ap()
    y = nc.dram_tensor("y", [N, D], BF16, kind="ExternalOutput").ap()

    with tile.TileContext(nc) as tc:
        _body(nc, tc, xt, wqkv, bqkv, wo, y)
    nc.compile()
    return nc


def _body(nc, tc, xt, wqkv, bqkv, wo, y):
    from contextlib import ExitStack

    with ExitStack() as ctx:
        const_pool = ctx.enter_context(tc.tile_pool(name="const", bufs=1))
        qkv_pool = ctx.enter_context(tc.tile_pool(name="qkv", bufs=1))
        exps_pool = ctx.enter_context(tc.tile_pool(name="exps", bufs=2))
        u_pool = ctx.enter_context(tc.tile_pool(name="u", bufs=2))
        ut_pool = ctx.enter_context(tc.tile_pool(name="ut", bufs=2))
        ao_pool = ctx.enter_context(tc.tile_pool(name="ao", bufs=2))
        aot_pool = ctx.enter_context(tc.tile_pool(name="aot", bufs=2))
        rec_pool = ctx.enter_context(tc.tile_pool(name="rec", bufs=4))
        y_pool = ctx.enter_context(tc.tile_pool(name="ysb", bufs=3))
        ps_big = ctx.enter_context(tc.tile_pool(name="psb", bufs=2, space="PSUM"))
        ps_sc = ctx.enter_context(tc.tile_pool(name="pssc", bufs=2, space="PSUM"))
        ps_u = ctx.enter_context(tc.tile_pool(name="psu", bufs=2, space="PSUM"))

        # DMA: x chunks ride the ACT DGE queue in parallel with weights on the
        # sync queue.  v weights come first (v is projected first so its XBAR
        # transpose to k-major hides under the q/k projections).
        xt_sb = const_pool.tile([128, 8, 4, 512], BF16)
        for i in range(4):
            nc.scalar.dma_start(out=xt_sb[:, :, i, :], in_=xt[i])
        bias_sb = const_pool.tile([128, 6], F32)
        nc.sync.dma_start(out=bias_sb[:], in_=bqkv[:])
        w_sb = const_pool.tile([128, 8, 768], BF16)
        nc.sync.dma_start(out=w_sb[:, :, 512:768], in_=wqkv[:, :, 512:768])
        nc.sync.dma_start(out=w_sb[:, :, 0:512], in_=wqkv[:, :, 0:512])
        wo_sb = const_pool.tile([128, 2, 1024], BF16)
        nc.sync.dma_start(out=wo_sb[:], in_=wo[:])

        qT = qkv_pool.tile([128, 2, N], BF16, tag="qT")
        kT = qkv_pool.tile([128, 2, N], BF16, tag="kT")
        vT = qkv_pool.tile([128, 2, N], BF16, tag="vT")
        # (k-token, head, kb, hd + ones col + zero pad to 80 for the XBAR)
        vaug = qkv_pool.tile([128, HPC, 16, 80], BF16, tag="vaug")
        nc.vector.memset(vaug[:, :, :, 64:65], 1.0)
        nc.vector.memset(vaug[:, :, :, 65:80], 0.0)

        # ---- Phase 1: QKV projection (e on partitions), v first ----
        for i in range(4):
            nsl = slice(i * 512, (i + 1) * 512)
            for et in (4, 5, 0, 1, 2, 3):  # v0 v1 q0 q1 k0 k1
                ps = ps_big.tile([128, 512], F32, tag="big", name=f"qkv_ps{i}_{et}")
                for dc in range(8):
                    nc.tensor.matmul(
                        ps[:],
                        w_sb[:, dc, et * 128 : (et + 1) * 128],
                        xt_sb[:, dc, i, :],
                        start=(dc == 0),
                        stop=(dc == 7),
                    )
                dst = (qT, qT, kT, kT, vT, vT)[et]
                nc.vector.tensor_scalar(
                    out=dst[:, et % 2, nsl], in0=ps[:],
                    scalar1=(0.125 if et < 2 else 1.0),
                    scalar2=bias_sb[:, et : et + 1],
                    op0=MUL, op1=ADD,
                )

        # V -> (k-token on partitions, hd) via DMA XBAR, one per head
        for h in range(HPC):
            hs = slice((h % 2) * 64, (h % 2) * 64 + 64)
            nc.sync.dma_start_transpose(
                out=vaug[:, h, :, 0:64], in_=vT[hs, h // 2, :]
            )

        # ---- Phase 2: attention + out-proj, per 512-token q-chunk ----
        # Out-proj for chunk i is emitted inside chunk i+1's head loop so the
        # PE never waits on the ao DMA-transposes.
        def emit_outproj(i, aoTb, qbs=range(4)):
            for qb in qbs:
                for dc in range(2):
                    dsl = slice(dc * 512, (dc + 1) * 512)
                    psy = ps_big.tile(
                        [128, 512], F32, tag="big", name=f"y_ps{i}_{qb}_{dc}"
                    )
                    nc.tensor.matmul(
                        psy[:],
                        aoTb[:, qb * 2 + 0, :],
                        wo_sb[:, 0, dsl],
                        start=True,
                        stop=False,
                    )
                    nc.tensor.matmul(
                        psy[:],
                        aoTb[:, qb * 2 + 1, :],
                        wo_sb[:, 1, dsl],
                        start=False,
                        stop=True,
                    )
                    ysb = y_pool.tile([128, 512], BF16, tag="ysb")
                    nc.vector.tensor_copy(out=ysb[:], in_=psy[:])
                    nc.sync.dma_start(
                        out=y[i * 512 + qb * 128 : i * 512 + (qb + 1) * 128, dsl],
                        in_=ysb[:],
                    )

        pending = None
        for i in range(4):
            qsl = slice(i * 512, (i + 1) * 512)
            aoq = ao_pool.tile([128, 4, 256], BF16, tag="aoq")  # (q, qb, e_loc)
            for h in range(HPC):
                hs = slice((h % 2) * 64, (h % 2) * 64 + 64)
                ho = h // 2
                exps = exps_pool.tile([128, 16, 512], BF16, tag="exps")
                for kp in range(8):
                    ps2 = ps_sc.tile([128, 2, 512], F32, tag="sc")
                    for j in range(2):
                        kb = kp * 2 + j
                        nc.tensor.matmul(
                            ps2[:, j, :],
                            kT[hs, ho, kb * 128 : (kb + 1) * 128],
                            qT[hs, ho, qsl],
                            start=True,
                            stop=True,
                        )
                    nc.scalar.activation(
                        exps[:, kp * 2 : kp * 2 + 2, :], ps2[:], AF.Exp
                    )
                psu = ps_u.tile([80, 512], F32, tag="psu")
                for kb in range(16):
                    nc.tensor.matmul(
                        psu[:],
                        vaug[:, h, kb, :],
                        exps[:, kb, :],
                        start=(kb == 0),
                        stop=(kb == 15),
                    )
                ub = u_pool.tile([80, 512], BF16, tag="ub")
                nc.vector.tensor_copy(out=ub[:], in_=psu[:])
                ubT = ut_pool.tile([128, 4, 80], BF16, tag="ubT")
                nc.sync.dma_start_transpose(out=ubT[:], in_=ub[:])
                rec = rec_pool.tile([128, 4], F32, tag="rec")
                nc.vector.reciprocal(out=rec[:], in_=ubT[:, :, 64])
                for qb in range(4):
                    nc.vector.tensor_scalar_mul(
                        aoq[:, qb, h * 64 : (h + 1) * 64],
                        ubT[:, qb, 0:64],
                        rec[:, qb : qb + 1],
                    )
                if h == 1 and pending is not None:
                    emit_outproj(*pending)
                    pending = None
            # attention out -> e-major via DMA XBAR, per 128-row block.  For
            # the last chunk, out-proj chases each block's transpose to keep
            # the tail shallow.
            aoTb = aot_pool.tile([128, 8, 128], BF16, tag="aoTb")
            for qb in range(4):
                nc.sync.dma_start_transpose(
                    out=aoTb[:, qb * 2 : qb * 2 + 2, :], in_=aoq[:, qb, :]
                )
                if i == 3:
                    emit_outproj(i, aoTb, qbs=(qb,))
            if i < 3:
                pending = (i, aoTb)


def make_in_maps(x, W_qkv, b_qkv, W_o):
    bf = ml_dtypes.bfloat16
    in_maps = []
    xt_b = []
    for b in range(B):
        x2 = np.asarray(x[b], dtype=np.float32)  # [2048, 1024]
        xt = x2.T.reshape(8, 128, N).transpose(1, 0, 2)
        xt = np.ascontiguousarray(
            xt.reshape(128, 8, 4, 512).transpose(2, 0, 1, 3)
        ).astype(bf)
        xt_b.append(xt)
    for c in range(NCORES):
        b, hg = c // 4, c % 4
        sl = slice(hg * 256, hg * 256 + 256)
        Wc = np.concatenate(
            [W_qkv[0:D][sl], W_qkv[D : 2 * D][sl], W_qkv[2 * D : 3 * D][sl]],
            axis=0,
        )  # [768, 1024]
        wc = np.ascontiguousarray(
            Wc.T.reshape(8, 128, 768).transpose(1, 0, 2)
        ).astype(bf)
        bc = np.concatenate(
            [b_qkv[0:D][sl] * 0.125, b_qkv[D : 2 * D][sl], b_qkv[2 * D : 3 * D][sl]]
        ).astype(np.float32)  # [768]
        bc = np.ascontiguousarray(bc.reshape(6, 128).T)  # [128, 6]
        woc = np.ascontiguousarray(
            W_o[:, sl].T.reshape(2, 128, 1024).transpose(1, 0, 2)
        ).astype(bf)  # [128, 2, 1024]
        in_maps.append({"xt": xt_b[b], "wqkv": wc, "bqkv": bc, "wo": woc})
    return in_maps


_NC_CACHE = {}


def get_nc():
    if "nc" not in _NC_CACHE:
        _NC_CACHE["nc"] = build_nc()
    return _NC_CACHE["nc"]


def run(inputs, trace=False, **kw):
    nc = get_nc()
    x = np.asarray(inputs["x"])
    W_qkv = np.asarray(inputs["W_qkv"], dtype=np.float32)
    b_qkv = np.asarray(inputs["b_qkv"], dtype=np.float32)
    W_o = np.asarray(inputs["W_o"], dtype=np.float32)
    b_o = np.asarray(inputs["b_o"], dtype=np.float32)
    in_maps = make_in_maps(x, W_qkv, b_qkv, W_o)
    res = run_bass_kernel_spmd(
        nc, in_maps, core_ids=list(range(NCORES)), trace=trace, **kw
    )
    parts = [np.asarray(m["y"], dtype=np.float32) for m in res.results]
    yb = []
    for b in range(B):
        yy = parts[4 * b]
        for g in range(1, 4):
            yy = yy + parts[4 * b + g]
        yb.append(yy)
    yout = np.stack(yb, axis=0) + b_o[None, None, :]
    return yout, res


def kernel(x, W_qkv, b_qkv, W_o, b_o):
    y, _ = run({"x": x, "W_qkv": W_qkv, "b_qkv": b_qkv, "W_o": W_o, "b_o": b_o})
    return y


# revision 14
# speedup vs baseline: 1.0749x; 1.0749x over previous
"""Distributed Trainium2 kernel for nn_AttentionBlock (B=2, N=2048, D=1024, H=16).

Sharding: 1 batch x 4 heads per core (batch = core//4, head group = core%4).
Each core computes QKV for its 4 heads over its batch's 2048 tokens, full
attention for those 4 (b, h) units, and a partial out-projection contracting
its 256 local attention-out dims.  The 4 partial [2048, 1024] products per
batch are summed on the host (the tensor-parallel all-reduce) along with b_o
and the v-bias correction W_o @ b_v (exact: attention rows sum to 1).
q/k biases are applied on device per-partition (k-bias cancels in softmax).

Per-core DRAM layouts:
  xt    [4, 128, 8, 512]  x[b]^T chunked by n-block (d_inner, d_outer, n) bf16
  wqkv  [128, 8, 768]     W_qkv rows for 4 heads, (d_inner, d_outer, e)   bf16
                          e-order: q0 q1 k0 k1 v0 v1 (128 each)
  bqkv  [128, 4]          bias for q0 q1 (pre-scaled 1/8) k0 k1           f32
  wo    [128, 2, 1024]    W_o[:, local cols]^T (e_inner, e_outer, d_out)  bf16
  y     [2048, 1024]      partial out-proj                                bf16

Pipeline: QKV q,k (e on partitions) -> V via flipped matmul directly into
(k, hd) layout with a ones column -> per (qc, head): scores (k, q) in pairs
of 512-col PSUM banks -> exp on ACT over 1024 cols -> PV with ones column
giving row sums -> bf16 PE-transpose (q on partitions) -> normalize by
1/sumexp -> bf16 PE-transpose back (e on partitions) -> out-proj -> y DMA,
interleaved per 128-row block so the tail is one block deep.  The bf16 PE
transposes interleaved with the big matmuls also keep the PE's power draw
below the hardware throttle threshold (pure matmul streams get capped to
50% utilization).
"""

import numpy as np
import ml_dtypes

import concourse.bass as bass
import concourse.tile as tile
from concourse import bacc, mybir
from concourse.bass_utils import run_bass_kernel_spmd
from concourse.masks import make_identity

B, N, D = 2, 2048, 1024
H, HD = 16, 64
NCORES = 8
HPC = 4  # heads per core

F32 = mybir.dt.float32
BF16 = mybir.dt.bfloat16
AF = mybir.ActivationFunctionType
MUL = mybir.AluOpType.mult
ADD = mybir.AluOpType.add


def build_nc():
    nc = bacc.Bacc(
        "TRN2", target_bir_lowering=False, debug=False, num_devices=NCORES
    )
    xt = nc.dram_tensor("xt", [4, 128, 8, 512], BF16, kind="ExternalInput").ap()
    wqkv = nc.dram_tensor("wqkv", [128, 8, 768], BF16, kind="ExternalInput").ap()
    bqkv = nc.dram_tensor("bqkv", [128, 4], F32, kind="ExternalInput").ap()
    wo = nc.dram_tensor("wo", [128, 2, 1024], BF16, kind="ExternalInput").ap()
    y = nc.dram_tensor("y", [N, D], BF16, kind="ExternalOutput").ap()

    with tile.TileContext(nc) as tc:
        _body(nc, tc, xt, wqkv, bqkv, wo, y)
    nc.compile()
    return nc


def _body(nc, tc, xt, wqkv, bqkv, wo, y):
    from contextlib import ExitStack

    with ExitStack() as ctx:
        const_pool = ctx.enter_context(tc.tile_pool(name="const", bufs=1))
        qkv_pool = ctx.enter_context(tc.tile_pool(name="qkv", bufs=1))
        exps_pool = ctx.enter_context(tc.tile_pool(name="exps", bufs=2))
        u_pool = ctx.enter_context(tc.tile_pool(name="u", bufs=2))
        ao_pool = ctx.enter_context(tc.tile_pool(name="ao", bufs=2))
        aot_pool = ctx.enter_context(tc.tile_pool(name="aot", bufs=2))
        rec_pool = ctx.enter_context(tc.tile_pool(name="rec", bufs=4))
        y_pool = ctx.enter_context(tc.tile_pool(name="ysb", bufs=3))
        ps_big = ctx.enter_context(tc.tile_pool(name="psb", bufs=2, space="PSUM"))
        ps_sc = ctx.enter_context(tc.tile_pool(name="pssc", bufs=2, space="PSUM"))
        ps_u = ctx.enter_context(tc.tile_pool(name="psu", bufs=1, space="PSUM"))
        ps_t = ctx.enter_context(tc.tile_pool(name="pst", bufs=1, space="PSUM"))

        # DMA: x chunks ride the ACT DGE queue in parallel with the weights
        # on the sync queue; the first QKV matmul needs bias+w_qk+xt0 only.
        xt_sb = const_pool.tile([128, 8, 4, 512], BF16)
        for i in range(4):
            nc.scalar.dma_start(out=xt_sb[:, :, i, :], in_=xt[i])
        bias_sb = const_pool.tile([128, 4], F32)
        nc.sync.dma_start(out=bias_sb[:], in_=bqkv[:])
        w_sb = const_pool.tile([128, 8, 768], BF16)
        nc.sync.dma_start(out=w_sb[:, :, 0:512], in_=wqkv[:, :, 0:512])
        nc.sync.dma_start(out=w_sb[:, :, 512:768], in_=wqkv[:, :, 512:768])
        wo_sb = const_pool.tile([128, 2, 1024], BF16)
        nc.sync.dma_start(out=wo_sb[:], in_=wo[:])
        ident = const_pool.tile([128, 128], BF16)
        make_identity(nc, ident[:])

        qT = qkv_pool.tile([128, 2, N], BF16, tag="qT")
        kT = qkv_pool.tile([128, 2, N], BF16, tag="kT")
        vaug = qkv_pool.tile([128, HPC, 16, 65], BF16, tag="vaug")
        nc.vector.memset(vaug[:, :, :, 64:65], 1.0)

        # ---- Phase 1a: Q, K projection (e on partitions) ----
        for i in range(4):
            nsl = slice(i * 512, (i + 1) * 512)
            for et in range(4):  # q0 q1 k0 k1
                ps = ps_big.tile([128, 512], F32, tag="big", name=f"qk_ps{i}_{et}")
                for dc in range(8):
                    nc.tensor.matmul(
                        ps[:],
                        w_sb[:, dc, et * 128 : (et + 1) * 128],
                        xt_sb[:, dc, i, :],
                        start=(dc == 0),
                        stop=(dc == 7),
                    )
                dst = qT if et < 2 else kT
                nc.vector.tensor_scalar(
                    out=dst[:, et % 2, nsl], in0=ps[:],
                    scalar1=(0.125 if et < 2 else 1.0),
                    scalar2=bias_sb[:, et : et + 1],
                    op0=MUL, op1=ADD,
                )

        # ---- Phase 1b: V projection, flipped (n on partitions) ----
        for nb in range(16):
            i, sub = nb // 4, nb % 4
            psv = ps_big.tile([128, 512], F32, tag="big", name=f"v_ps{nb}")
            for dc in range(8):
                nc.tensor.matmul(
                    psv[:, 0:256],
                    xt_sb[:, dc, i, sub * 128 : (sub + 1) * 128],
                    w_sb[:, dc, 512:768],
                    start=(dc == 0),
                    stop=(dc == 7),
                )
            nc.vector.tensor_copy(
                out=vaug[:, :, nb, 0:64],
                in_=psv[:, 0:256].rearrange("p (h d) -> p h d", h=HPC),
            )

        # ---- Phase 2: attention + out-proj, per 512-token q-chunk ----
        for i in range(4):
            qsl = slice(i * 512, (i + 1) * 512)
            aoq = ao_pool.tile([128, 4, 256], BF16, tag="aoq")  # (q, qb, e_loc)
            for h in range(HPC):
                hs = slice((h % 2) * 64, (h % 2) * 64 + 64)
                ho = h // 2
                exps = exps_pool.tile([128, 16, 512], BF16, tag="exps")
                for kp in range(8):
                    ps2 = ps_sc.tile([128, 2, 512], F32, tag="sc")
                    for j in range(2):
                        kb = kp * 2 + j
                        nc.tensor.matmul(
                            ps2[:, j, :],
                            kT[hs, ho, kb * 128 : (kb + 1) * 128],
                            qT[hs, ho, qsl],
                            start=True,
                            stop=True,
                        )
                    nc.scalar.activation(
                        exps[:, kp * 2 : kp * 2 + 2, :], ps2[:], AF.Exp
                    )
                psu = ps_u.tile([65, 512], F32, tag="psu")
                for kb in range(16):
                    nc.tensor.matmul(
                        psu[:],
                        vaug[:, h, kb, :],
                        exps[:, kb, :],
                        start=(kb == 0),
                        stop=(kb == 15),
                    )
                ub = u_pool.tile([65, 512], BF16, tag="ub")
                nc.vector.tensor_copy(out=ub[:], in_=psu[:])
                pst = ps_t.tile([128, 8, 128], BF16, tag="pst")
                for qb in range(4):
                    nc.tensor.transpose(
                        pst[:, qb, 0:65], ub[:, qb * 128 : (qb + 1) * 128],
                        ident[0:65, 0:65],
                    )
                rec = rec_pool.tile([128, 4], F32, tag="rec")
                nc.vector.reciprocal(out=rec[:], in_=pst[:, 0:4, 64])
                for qb in range(4):
                    nc.vector.tensor_scalar_mul(
                        aoq[:, qb, h * 64 : (h + 1) * 64],
                        pst[:, qb, 0:64],
                        rec[:, qb : qb + 1],
                    )
            # transpose back (e on partitions) and out-project, per 128-row
            # block so the pipeline tail is one block deep
            pstT = ps_t.tile([128, 8, 128], BF16, tag="pst")
            for qb in range(4):
                aoT = aot_pool.tile([128, 2, 128], BF16, tag="aoT")
                for eo in range(2):
                    nc.tensor.transpose(
                        pstT[:, qb * 2 + eo, :],
                        aoq[:, qb, eo * 128 : (eo + 1) * 128],
                        ident[:],
                    )
                nc.vector.tensor_copy(
                    out=aoT[:], in_=pstT[:, qb * 2 : qb * 2 + 2, :]
                )
                for dc in range(2):
                    dsl = slice(dc * 512, (dc + 1) * 512)
                    psy = ps_big.tile([128, 512], F32, tag="big", name=f"y_ps{i}_{qb}_{dc}")
                    nc.tensor.matmul(
                        psy[:],
                        aoT[:, 0, :],
                        wo_sb[:, 0, dsl],
                        start=True,
                        stop=False,
                    )
                    nc.tensor.matmul(
                        psy[:],
                        aoT[:, 1, :],
                        wo_sb[:, 1, dsl],
                        start=False,
                        stop=True,
                    )
                    ysb = y_pool.tile([128, 512], BF16, tag="ysb")
                    nc.vector.tensor_copy(out=ysb[:], in_=psy[:])
                    nc.sync.dma_start(
                        out=y[i * 512 + qb * 128 : i * 512 + (qb + 1) * 128, dsl],
                        in_=ysb[:],
                    )


def make_in_maps(x, W_qkv, b_qkv, W_o):
    bf = ml_dtypes.bfloat16
    in_maps = []
    xt_b = []
    for b in range(B):
        x2 = np.asarray(x[b], dtype=np.float32)  # [2048, 1024]
        xt = x2.T.reshape(8, 128, N).transpose(1, 0, 2)
        xt = np.ascontiguousarray(
            xt.reshape(128, 8, 4, 512).transpose(2, 0, 1, 3)
        ).astype(bf)
        xt_b.append(xt)
    for c in range(NCORES):
        b, hg = c // 4, c % 4
        sl = slice(hg * 256, hg * 256 + 256)
        Wc = np.concatenate(
            [W_qkv[0:D][sl], W_qkv[D : 2 * D][sl], W_qkv[2 * D : 3 * D][sl]],
            axis=0,
        )  # [768, 1024]
        wc = np.ascontiguousarray(
            Wc.T.reshape(8, 128, 768).transpose(1, 0, 2)
        ).astype(bf)
        bc = np.concatenate(
            [b_qkv[0:D][sl] * 0.125, b_qkv[D : 2 * D][sl]]
        ).astype(np.float32)  # [512]
        bc = np.ascontiguousarray(bc.reshape(4, 128).T)  # [128, 4]
        woc = np.ascontiguousarray(
            W_o[:, sl].T.reshape(2, 128, 1024).transpose(1, 0, 2)
        ).astype(bf)  # [128, 2, 1024]
        in_maps.append({"xt": xt_b[b], "wqkv": wc, "bqkv": bc, "wo": woc})
    return in_maps


_NC_CACHE = {}


def get_nc():
    if "nc" not in _NC_CACHE:
        _NC_CACHE["nc"] = build_nc()
    return _NC_CACHE["nc"]


def run(inputs, trace=False, **kw):
    nc = get_nc()
    x = np.asarray(inputs["x"])
    W_qkv = np.asarray(inputs["W_qkv"], dtype=np.float32)
    b_qkv = np.asarray(inputs["b_qkv"], dtype=np.float32)
    W_o = np.asarray(inputs["W_o"], dtype=np.float32)
    b_o = np.asarray(inputs["b_o"], dtype=np.float32)
    in_maps = make_in_maps(x, W_qkv, b_qkv, W_o)
    res = run_bass_kernel_spmd(
        nc, in_maps, core_ids=list(range(NCORES)), trace=trace, **kw
    )
    parts = [np.asarray(m["y"], dtype=np.float32) for m in res.results]
    yb = []
    for b in range(B):
        yy = parts[4 * b]
        for g in range(1, 4):
            yy = yy + parts[4 * b + g]
        yb.append(yy)
    yout = np.stack(yb, axis=0)
    # exact bias correction: v-bias flows through attention (rows sum to 1)
    # into out-proj; k-bias cancels in softmax; q-bias applied on device.
    corr = W_o @ b_qkv[2 * D : 3 * D] + b_o
    yout = yout + corr[None, None, :]
    return yout, res


def kernel(x, W_qkv, b_qkv, W_o, b_o):
    y, _ = run({"x": x, "W_qkv": W_qkv, "b_qkv": b_qkv, "W_o": W_o, "b_o": b_o})
    return y


# revision 17
# speedup vs baseline: 1.0900x; 1.0141x over previous
"""Distributed Trainium2 kernel for nn_AttentionBlock (B=2, N=2048, D=1024, H=16).

Sharding: 1 batch x 4 heads per core (batch = core//4, head group = core%4).
Each core computes QKV for its 4 heads over its batch's 2048 tokens, full
attention for those 4 (b, h) units, and a partial out-projection contracting
its 256 local attention-out dims.  The 4 partial [2048, 1024] products per
batch are summed on the host (the tensor-parallel all-reduce) along with b_o
and the v-bias correction W_o @ b_v (exact: attention rows sum to 1).
q/k biases are applied on device per-partition (k-bias cancels in softmax).

Per-core DRAM layouts:
  xt    [4, 128, 8, 512]  x[b]^T chunked by n-block (d_inner, d_outer, n) bf16
  wqkv  [128, 8, 768]     W_qkv rows for 4 heads, (d_inner, d_outer, e)   bf16
                          e-order: q0 q1 k0 k1 v0 v1 (128 each)
  bqkv  [128, 4]          bias for q0 q1 (pre-scaled 1/8) k0 k1           f32
  wo    [128, 2, 1024]    W_o[:, local cols]^T (e_inner, e_outer, d_out)  bf16
  y     [2048, 1024]      partial out-proj                                bf16

Pipeline: QKV q,k (e on partitions) -> V via flipped matmul directly into
(k, hd) layout with a ones column -> per (qc, head): scores (k, q) in pairs
of 512-col PSUM banks -> exp on ACT over 1024 cols -> PV with ones column
giving row sums -> bf16 PE-transpose (q on partitions) -> normalize by
1/sumexp -> bf16 PE-transpose back (e on partitions) -> out-proj -> y DMA,
interleaved per 128-row block so the tail is one block deep.  The bf16 PE
transposes interleaved with the big matmuls also keep the PE's power draw
below the hardware throttle threshold (pure matmul streams get capped to
50% utilization).
"""

import numpy as np
import ml_dtypes

import concourse.bass as bass
import concourse.tile as tile
from concourse import bacc, mybir
from concourse.bass_utils import run_bass_kernel_spmd
from concourse.masks import make_identity

B, N, D = 2, 2048, 1024
H, HD = 16, 64
NCORES = 8
HPC = 4  # heads per core

F32 = mybir.dt.float32
BF16 = mybir.dt.bfloat16
AF = mybir.ActivationFunctionType
MUL = mybir.AluOpType.mult
ADD = mybir.AluOpType.add


def build_nc():
    nc = bacc.Bacc(
        "TRN2", target_bir_lowering=False, debug=False, num_devices=NCORES
    )
    xt = nc.dram_tensor("xt", [4, 128, 8, 512], BF16, kind="ExternalInput").ap()
    wqkv = nc.dram_tensor("wqkv", [128, 8, 768], BF16, kind="ExternalInput").ap()
    bqkv = nc.dram_tensor("bqkv", [128, 4], F32, kind="ExternalInput").ap()
    wo = nc.dram_tensor("wo", [128, 2, 1024], BF16, kind="ExternalInput").ap()
    y = nc.dram_tensor("y", [N, D], BF16, kind="ExternalOutput").ap()

    with tile.TileContext(nc) as tc:
        _body(nc, tc, xt, wqkv, bqkv, wo, y)
    nc.compile()
    return nc


def _body(nc, tc, xt, wqkv, bqkv, wo, y):
    from contextlib import ExitStack

    with ExitStack() as ctx:
        const_pool = ctx.enter_context(tc.tile_pool(name="const", bufs=1))
        qkv_pool = ctx.enter_context(tc.tile_pool(name="qkv", bufs=1))
        exps_pool = ctx.enter_context(tc.tile_pool(name="exps", bufs=2))
        u_pool = ctx.enter_context(tc.tile_pool(name="u", bufs=2))
        ao_pool = ctx.enter_context(tc.tile_pool(name="ao", bufs=2))
        aot_pool = ctx.enter_context(tc.tile_pool(name="aot", bufs=2))
        rec_pool = ctx.enter_context(tc.tile_pool(name="rec", bufs=4))
        y_pool = ctx.enter_context(tc.tile_pool(name="ysb", bufs=3))
        ps_big = ctx.enter_context(tc.tile_pool(name="psb", bufs=2, space="PSUM"))
        ps_sc = ctx.enter_context(tc.tile_pool(name="pssc", bufs=2, space="PSUM"))
        ps_u = ctx.enter_context(tc.tile_pool(name="psu", bufs=1, space="PSUM"))
        ps_t = ctx.enter_context(tc.tile_pool(name="pst", bufs=1, space="PSUM"))

        # DMA: the first QKV matmul needs only bias + q/k weights + xt0, so
        # those go first; v weights issue after phase 1a and W_o after phase
        # 1b (in-order queue: later issue keeps HBM bandwidth on the head).
        bias_sb = const_pool.tile([128, 4], F32)
        nc.sync.dma_start(out=bias_sb[:], in_=bqkv[:])
        w_sb = const_pool.tile([128, 8, 768], BF16)
        nc.sync.dma_start(out=w_sb[:, :, 0:512], in_=wqkv[:, :, 0:512])
        xt_sb = const_pool.tile([128, 8, 4, 512], BF16)
        for i in range(4):
            nc.sync.dma_start(out=xt_sb[:, :, i, :], in_=xt[i])
        wo_sb = const_pool.tile([128, 2, 1024], BF16)
        ident = const_pool.tile([128, 128], BF16)
        make_identity(nc, ident[:])

        qT = qkv_pool.tile([128, 2, N], BF16, tag="qT")
        kT = qkv_pool.tile([128, 2, N], BF16, tag="kT")
        vaug = qkv_pool.tile([128, HPC, 16, 65], BF16, tag="vaug")
        nc.vector.memset(vaug[:, :, :, 64:65], 1.0)

        # ---- Phase 1a: Q, K projection (e on partitions) ----
        for i in range(4):
            nsl = slice(i * 512, (i + 1) * 512)
            for et in range(4):  # q0 q1 k0 k1
                ps = ps_big.tile([128, 512], F32, tag="big", name=f"qk_ps{i}_{et}")
                for dc in range(8):
                    nc.tensor.matmul(
                        ps[:],
                        w_sb[:, dc, et * 128 : (et + 1) * 128],
                        xt_sb[:, dc, i, :],
                        start=(dc == 0),
                        stop=(dc == 7),
                    )
                dst = qT if et < 2 else kT
                nc.vector.tensor_scalar(
                    out=dst[:, et % 2, nsl], in0=ps[:],
                    scalar1=(0.125 if et < 2 else 1.0),
                    scalar2=bias_sb[:, et : et + 1],
                    op0=MUL, op1=ADD,
                )

        nc.sync.dma_start(out=w_sb[:, :, 512:768], in_=wqkv[:, :, 512:768])

        # ---- Phase 1b: V projection, flipped (n on partitions) ----
        for nb in range(16):
            i, sub = nb // 4, nb % 4
            psv = ps_big.tile([128, 512], F32, tag="big", name=f"v_ps{nb}")
            for dc in range(8):
                nc.tensor.matmul(
                    psv[:, 0:256],
                    xt_sb[:, dc, i, sub * 128 : (sub + 1) * 128],
                    w_sb[:, dc, 512:768],
                    start=(dc == 0),
                    stop=(dc == 7),
                )
            nc.vector.tensor_copy(
                out=vaug[:, :, nb, 0:64],
                in_=psv[:, 0:256].rearrange("p (h d) -> p h d", h=HPC),
            )

        nc.sync.dma_start(out=wo_sb[:], in_=wo[:])

        # ---- Phase 2: attention + out-proj, per 512-token q-chunk ----
        for i in range(4):
            qsl = slice(i * 512, (i + 1) * 512)
            aoq = ao_pool.tile([128, 4, 256], BF16, tag="aoq")  # (q, qb, e_loc)
            for h in range(HPC):
                hs = slice((h % 2) * 64, (h % 2) * 64 + 64)
                ho = h // 2
                exps = exps_pool.tile([128, 16, 512], BF16, tag="exps")
                for kp in range(8):
                    ps2 = ps_sc.tile([128, 2, 512], F32, tag="sc")
                    for j in range(2):
                        kb = kp * 2 + j
                        nc.tensor.matmul(
                            ps2[:, j, :],
                            kT[hs, ho, kb * 128 : (kb + 1) * 128],
                            qT[hs, ho, qsl],
                            start=True,
                            stop=True,
                        )
                    nc.scalar.activation(
                        exps[:, kp * 2 : kp * 2 + 2, :], ps2[:], AF.Exp
                    )
                psu = ps_u.tile([65, 512], F32, tag="psu")
                for kb in range(16):
                    nc.tensor.matmul(
                        psu[:],
                        vaug[:, h, kb, :],
                        exps[:, kb, :],
                        start=(kb == 0),
                        stop=(kb == 15),
                    )
                ub = u_pool.tile([65, 512], BF16, tag="ub")
                nc.vector.tensor_copy(out=ub[:], in_=psu[:])
                pst = ps_t.tile([128, 8, 128], BF16, tag="pst")
                for qb in range(4):
                    nc.tensor.transpose(
                        pst[:, qb, 0:65], ub[:, qb * 128 : (qb + 1) * 128],
                        ident[0:65, 0:65],
                    )
                rec = rec_pool.tile([128, 4], F32, tag="rec")
                nc.vector.reciprocal(out=rec[:], in_=pst[:, 0:4, 64])
                for qb in range(4):
                    nc.vector.tensor_scalar_mul(
                        aoq[:, qb, h * 64 : (h + 1) * 64],
                        pst[:, qb, 0:64],
                        rec[:, qb : qb + 1],
                    )
            # transpose back (e on partitions) and out-project, per 128-row
            # block so the pipeline tail is one block deep
            pstT = ps_t.tile([128, 8, 128], BF16, tag="pst")
            for qb in range(4):
                aoT = aot_pool.tile([128, 2, 128], BF16, tag="aoT")
                for eo in range(2):
                    nc.tensor.transpose(
                        pstT[:, qb * 2 + eo, :],
                        aoq[:, qb, eo * 128 : (eo + 1) * 128],
                        ident[:],
                    )
                nc.vector.tensor_copy(
                    out=aoT[:], in_=pstT[:, qb * 2 : qb * 2 + 2, :]
                )
                for dc in range(2):
                    dsl = slice(dc * 512, (dc + 1) * 512)
                    psy = ps_big.tile([128, 512], F32, tag="big", name=f"y_ps{i}_{qb}_{dc}")
                    nc.tensor.matmul(
                        psy[:],
                        aoT[:, 0, :],
                        wo_sb[:, 0, dsl],
                        start=True,
                        stop=False,
                    )
                    nc.tensor.matmul(
                        psy[:],
                        aoT[:, 1, :],
                        wo_sb[:, 1, dsl],
                        start=False,
                        stop=True,
                    )
                    ysb = y_pool.tile([128, 512], BF16, tag="ysb")
                    nc.vector.tensor_copy(out=ysb[:], in_=psy[:])
                    nc.sync.dma_start(
                        out=y[i * 512 + qb * 128 : i * 512 + (qb + 1) * 128, dsl],
                        in_=ysb[:],
                    )


def make_in_maps(x, W_qkv, b_qkv, W_o):
    bf = ml_dtypes.bfloat16
    in_maps = []
    xt_b = []
    for b in range(B):
        x2 = np.asarray(x[b], dtype=np.float32)  # [2048, 1024]
        xt = x2.T.reshape(8, 128, N).transpose(1, 0, 2)
        xt = np.ascontiguousarray(
            xt.reshape(128, 8, 4, 512).transpose(2, 0, 1, 3)
        ).astype(bf)
        xt_b.append(xt)
    for c in range(NCORES):
        b, hg = c // 4, c % 4
        sl = slice(hg * 256, hg * 256 + 256)
        Wc = np.concatenate(
            [W_qkv[0:D][sl], W_qkv[D : 2 * D][sl], W_qkv[2 * D : 3 * D][sl]],
            axis=0,
        )  # [768, 1024]
        wc = np.ascontiguousarray(
            Wc.T.reshape(8, 128, 768).transpose(1, 0, 2)
        ).astype(bf)
        bc = np.concatenate(
            [b_qkv[0:D][sl] * 0.125, b_qkv[D : 2 * D][sl]]
        ).astype(np.float32)  # [512]
        bc = np.ascontiguousarray(bc.reshape(4, 128).T)  # [128, 4]
        woc = np.ascontiguousarray(
            W_o[:, sl].T.reshape(2, 128, 1024).transpose(1, 0, 2)
        ).astype(bf)  # [128, 2, 1024]
        in_maps.append({"xt": xt_b[b], "wqkv": wc, "bqkv": bc, "wo": woc})
    return in_maps


_NC_CACHE = {}


def get_nc():
    if "nc" not in _NC_CACHE:
        _NC_CACHE["nc"] = build_nc()
    return _NC_CACHE["nc"]


def run(inputs, trace=False, **kw):
    nc = get_nc()
    x = np.asarray(inputs["x"])
    W_qkv = np.asarray(inputs["W_qkv"], dtype=np.float32)
    b_qkv = np.asarray(inputs["b_qkv"], dtype=np.float32)
    W_o = np.asarray(inputs["W_o"], dtype=np.float32)
    b_o = np.asarray(inputs["b_o"], dtype=np.float32)
    in_maps = make_in_maps(x, W_qkv, b_qkv, W_o)
    res = run_bass_kernel_spmd(
        nc, in_maps, core_ids=list(range(NCORES)), trace=trace, **kw
    )
    parts = [np.asarray(m["y"], dtype=np.float32) for m in res.results]
    yb = []
    for b in range(B):
        yy = parts[4 * b]
        for g in range(1, 4):
            yy = yy + parts[4 * b + g]
        yb.append(yy)
    yout = np.stack(yb, axis=0)
    # exact bias correction: v-bias flows through attention (rows sum to 1)
    # into out-proj; k-bias cancels in softmax; q-bias applied on device.
    corr = W_o @ b_qkv[2 * D : 3 * D] + b_o
    yout = yout + corr[None, None, :]
    return yout, res


def kernel(x, W_qkv, b_qkv, W_o, b_o):
    y, _ = run({"x": x, "W_qkv": W_qkv, "b_qkv": b_qkv, "W_o": W_o, "b_o": b_o})
    return y


# revision 18
# speedup vs baseline: 1.0995x; 1.0087x over previous
"""Distributed Trainium2 kernel for nn_AttentionBlock (B=2, N=2048, D=1024, H=16).

Sharding: 1 batch x 4 heads per core (batch = core//4, head group = core%4).
Each core computes QKV for its 4 heads over its batch's 2048 tokens, full
attention for those 4 (b, h) units, and a partial out-projection contracting
its 256 local attention-out dims.  The 4 partial [2048, 1024] products per
batch are summed on the host (the tensor-parallel all-reduce) along with b_o
and the v-bias correction W_o @ b_v (exact: attention rows sum to 1).
q/k biases are applied on device per-partition (k-bias cancels in softmax).

Per-core DRAM layouts:
  xt    [4, 128, 8, 512]  x[b]^T chunked by n-block (d_inner, d_outer, n) bf16
  wqkv  [128, 8, 768]     W_qkv rows for 4 heads, (d_inner, d_outer, e)   bf16
                          e-order: q0 q1 k0 k1 v0 v1 (128 each)
  bqkv  [128, 4]          bias for q0 q1 (pre-scaled 1/8) k0 k1           f32
  wo    [128, 2, 1024]    W_o[:, local cols]^T (e_inner, e_outer, d_out)  bf16
  y     [2048, 1024]      partial out-proj                                bf16

Pipeline: QKV q,k (e on partitions) -> V via flipped matmul directly into
(k, hd) layout with a ones column -> per (qc, head): scores (k, q) in pairs
of 512-col PSUM banks -> exp on ACT over 1024 cols -> PV with ones column
giving row sums -> bf16 PE-transpose (q on partitions) -> normalize by
1/sumexp -> bf16 PE-transpose back (e on partitions) -> out-proj -> y DMA,
interleaved per 128-row block so the tail is one block deep.  The bf16 PE
transposes interleaved with the big matmuls also keep the PE's power draw
below the hardware throttle threshold (pure matmul streams get capped to
50% utilization).
"""

import numpy as np
import ml_dtypes

import concourse.bass as bass
import concourse.tile as tile
from concourse import bacc, mybir
from concourse.bass_utils import run_bass_kernel_spmd
from concourse.masks import make_identity

B, N, D = 2, 2048, 1024
H, HD = 16, 64
NCORES = 8
HPC = 4  # heads per core

F32 = mybir.dt.float32
BF16 = mybir.dt.bfloat16
AF = mybir.ActivationFunctionType
MUL = mybir.AluOpType.mult
ADD = mybir.AluOpType.add


def build_nc():
    nc = bacc.Bacc(
        "TRN2", target_bir_lowering=False, debug=False, num_devices=NCORES
    )
    xt = nc.dram_tensor("xt", [4, 128, 8, 512], BF16, kind="ExternalInput").ap()
    wqkv = nc.dram_tensor("wqkv", [128, 8, 768], BF16, kind="ExternalInput").ap()
    bqkv = nc.dram_tensor("bqkv", [128, 4], F32, kind="ExternalInput").ap()
    wo = nc.dram_tensor("wo", [128, 2, 1024], BF16, kind="ExternalInput").ap()
    y = nc.dram_tensor("y", [N, D], BF16, kind="ExternalOutput").ap()

    with tile.TileContext(nc) as tc:
        _body(nc, tc, xt, wqkv, bqkv, wo, y)
    nc.compile()
    return nc


def _body(nc, tc, xt, wqkv, bqkv, wo, y):
    from contextlib import ExitStack

    with ExitStack() as ctx:
        const_pool = ctx.enter_context(tc.tile_pool(name="const", bufs=1))
        qkv_pool = ctx.enter_context(tc.tile_pool(name="qkv", bufs=1))
        exps_pool = ctx.enter_context(tc.tile_pool(name="exps", bufs=2))
        u_pool = ctx.enter_context(tc.tile_pool(name="u", bufs=2))
        ao_pool = ctx.enter_context(tc.tile_pool(name="ao", bufs=2))
        aot_pool = ctx.enter_context(tc.tile_pool(name="aot", bufs=2))
        rec_pool = ctx.enter_context(tc.tile_pool(name="rec", bufs=4))
        y_pool = ctx.enter_context(tc.tile_pool(name="ysb", bufs=3))
        ps_big = ctx.enter_context(tc.tile_pool(name="psb", bufs=2, space="PSUM"))
        ps_sc = ctx.enter_context(tc.tile_pool(name="pssc", bufs=2, space="PSUM"))
        ps_u = ctx.enter_context(tc.tile_pool(name="psu", bufs=1, space="PSUM"))
        ps_t = ctx.enter_context(tc.tile_pool(name="pst", bufs=1, space="PSUM"))

        # DMA: the first QKV matmuls need only the q weights and the first
        # half of xt0, so those transfer first in small pieces; the bias is
        # only needed at the first PSUM drain.  v weights issue after phase
        # 1a and W_o after phase 1b (in-order queue: later issue keeps HBM
        # bandwidth on the head).
        w_sb = const_pool.tile([128, 8, 768], BF16)
        nc.sync.dma_start(out=w_sb[:, :, 0:256], in_=wqkv[:, :, 0:256])
        xt_sb = const_pool.tile([128, 8, 4, 512], BF16)
        nc.sync.dma_start(out=xt_sb[:, 0:4, 0, :], in_=xt[0][:, 0:4])
        nc.sync.dma_start(out=xt_sb[:, 4:8, 0, :], in_=xt[0][:, 4:8])
        bias_sb = const_pool.tile([128, 4], F32)
        nc.sync.dma_start(out=bias_sb[:], in_=bqkv[:])
        nc.sync.dma_start(out=w_sb[:, :, 256:512], in_=wqkv[:, :, 256:512])
        for i in range(1, 4):
            nc.sync.dma_start(out=xt_sb[:, :, i, :], in_=xt[i])
        wo_sb = const_pool.tile([128, 2, 1024], BF16)
        ident = const_pool.tile([128, 128], BF16)
        make_identity(nc, ident[:])

        qT = qkv_pool.tile([128, 2, N], BF16, tag="qT")
        kT = qkv_pool.tile([128, 2, N], BF16, tag="kT")
        vaug = qkv_pool.tile([128, HPC, 16, 65], BF16, tag="vaug")
        nc.vector.memset(vaug[:, :, :, 64:65], 1.0)

        # ---- Phase 1a: Q, K projection (e on partitions) ----
        for i in range(4):
            nsl = slice(i * 512, (i + 1) * 512)
            for et in range(4):  # q0 q1 k0 k1
                ps = ps_big.tile([128, 512], F32, tag="big", name=f"qk_ps{i}_{et}")
                for dc in range(8):
                    nc.tensor.matmul(
                        ps[:],
                        w_sb[:, dc, et * 128 : (et + 1) * 128],
                        xt_sb[:, dc, i, :],
                        start=(dc == 0),
                        stop=(dc == 7),
                    )
                dst = qT if et < 2 else kT
                nc.vector.tensor_scalar(
                    out=dst[:, et % 2, nsl], in0=ps[:],
                    scalar1=(0.125 if et < 2 else 1.0),
                    scalar2=bias_sb[:, et : et + 1],
                    op0=MUL, op1=ADD,
                )

        nc.sync.dma_start(out=w_sb[:, :, 512:768], in_=wqkv[:, :, 512:768])

        # ---- Phase 1b: V projection, flipped (n on partitions) ----
        for nb in range(16):
            i, sub = nb // 4, nb % 4
            psv = ps_big.tile([128, 512], F32, tag="big", name=f"v_ps{nb}")
            for dc in range(8):
                nc.tensor.matmul(
                    psv[:, 0:256],
                    xt_sb[:, dc, i, sub * 128 : (sub + 1) * 128],
                    w_sb[:, dc, 512:768],
                    start=(dc == 0),
                    stop=(dc == 7),
                )
            nc.vector.tensor_copy(
                out=vaug[:, :, nb, 0:64],
                in_=psv[:, 0:256].rearrange("p (h d) -> p h d", h=HPC),
            )

        nc.sync.dma_start(out=wo_sb[:], in_=wo[:])

        # ---- Phase 2: attention + out-proj, per 512-token q-chunk ----
        for i in range(4):
            qsl = slice(i * 512, (i + 1) * 512)
            aoq = ao_pool.tile([128, 4, 256], BF16, tag="aoq")  # (q, qb, e_loc)
            for h in range(HPC):
                hs = slice((h % 2) * 64, (h % 2) * 64 + 64)
                ho = h // 2
                exps = exps_pool.tile([128, 16, 512], BF16, tag="exps")
                for kp in range(8):
                    ps2 = ps_sc.tile([128, 2, 512], F32, tag="sc")
                    for j in range(2):
                        kb = kp * 2 + j
                        nc.tensor.matmul(
                            ps2[:, j, :],
                            kT[hs, ho, kb * 128 : (kb + 1) * 128],
                            qT[hs, ho, qsl],
                            start=True,
                            stop=True,
                        )
                    nc.scalar.activation(
                        exps[:, kp * 2 : kp * 2 + 2, :], ps2[:], AF.Exp
                    )
                psu = ps_u.tile([65, 512], F32, tag="psu")
                for kb in range(16):
                    nc.tensor.matmul(
                        psu[:],
                        vaug[:, h, kb, :],
                        exps[:, kb, :],
                        start=(kb == 0),
                        stop=(kb == 15),
                    )
                ub = u_pool.tile([65, 512], BF16, tag="ub")
                nc.vector.tensor_copy(out=ub[:], in_=psu[:])
                pst = ps_t.tile([128, 8, 128], BF16, tag="pst")
                for qb in range(4):
                    nc.tensor.transpose(
                        pst[:, qb, 0:65], ub[:, qb * 128 : (qb + 1) * 128],
                        ident[0:65, 0:65],
                    )
                rec = rec_pool.tile([128, 4], F32, tag="rec")
                nc.vector.reciprocal(out=rec[:], in_=pst[:, 0:4, 64])
                for qb in range(4):
                    nc.vector.tensor_scalar_mul(
                        aoq[:, qb, h * 64 : (h + 1) * 64],
                        pst[:, qb, 0:64],
                        rec[:, qb : qb + 1],
                    )
            # transpose back (e on partitions) and out-project, per 128-row
            # block so the pipeline tail is one block deep
            pstT = ps_t.tile([128, 8, 128], BF16, tag="pst")
            for qb in range(4):
                aoT = aot_pool.tile([128, 2, 128], BF16, tag="aoT")
                for eo in range(2):
                    nc.tensor.transpose(
                        pstT[:, qb * 2 + eo, :],
                        aoq[:, qb, eo * 128 : (eo + 1) * 128],
                        ident[:],
                    )
                nc.vector.tensor_copy(
                    out=aoT[:], in_=pstT[:, qb * 2 : qb * 2 + 2, :]
                )
                for dc in range(2):
                    dsl = slice(dc * 512, (dc + 1) * 512)
                    psy = ps_big.tile([128, 512], F32, tag="big", name=f"y_ps{i}_{qb}_{dc}")
                    nc.tensor.matmul(
                        psy[:],
                        aoT[:, 0, :],
                        wo_sb[:, 0, dsl],
                        start=True,
                        stop=False,
                    )
                    nc.tensor.matmul(
                        psy[:],
                        aoT[:, 1, :],
                        wo_sb[:, 1, dsl],
                        start=False,
                        stop=True,
                    )
                    ysb = y_pool.tile([128, 512], BF16, tag="ysb")
                    nc.vector.tensor_copy(out=ysb[:], in_=psy[:])
                    nc.sync.dma_start(
                        out=y[i * 512 + qb * 128 : i * 512 + (qb + 1) * 128, dsl],
                        in_=ysb[:],
                    )


def make_in_maps(x, W_qkv, b_qkv, W_o):
    bf = ml_dtypes.bfloat16
    in_maps = []
    xt_b = []
    for b in range(B):
        x2 = np.asarray(x[b], dtype=np.float32)  # [2048, 1024]
        xt = x2.T.reshape(8, 128, N).transpose(1, 0, 2)
        xt = np.ascontiguousarray(
            xt.reshape(128, 8, 4, 512).transpose(2, 0, 1, 3)
        ).astype(bf)
        xt_b.append(xt)
    for c in range(NCORES):
        b, hg = c // 4, c % 4
        sl = slice(hg * 256, hg * 256 + 256)
        Wc = np.concatenate(
            [W_qkv[0:D][sl], W_qkv[D : 2 * D][sl], W_qkv[2 * D : 3 * D][sl]],
            axis=0,
        )  # [768, 1024]
        wc = np.ascontiguousarray(
            Wc.T.reshape(8, 128, 768).transpose(1, 0, 2)
        ).astype(bf)
        bc = np.concatenate(
            [b_qkv[0:D][sl] * 0.125, b_qkv[D : 2 * D][sl]]
        ).astype(np.float32)  # [512]
        bc = np.ascontiguousarray(bc.reshape(4, 128).T)  # [128, 4]
        woc = np.ascontiguousarray(
            W_o[:, sl].T.reshape(2, 128, 1024).transpose(1, 0, 2)
        ).astype(bf)  # [128, 2, 1024]
        in_maps.append({"xt": xt_b[b], "wqkv": wc, "bqkv": bc, "wo": woc})
    return in_maps


_NC_CACHE = {}


def get_nc():
    if "nc" not in _NC_CACHE:
        _NC_CACHE["nc"] = build_nc()
    return _NC_CACHE["nc"]


def run(inputs, trace=False, **kw):
    nc = get_nc()
    x = np.asarray(inputs["x"])
    W_qkv = np.asarray(inputs["W_qkv"], dtype=np.float32)
    b_qkv = np.asarray(inputs["b_qkv"], dtype=np.float32)
    W_o = np.asarray(inputs["W_o"], dtype=np.float32)
    b_o = np.asarray(inputs["b_o"], dtype=np.float32)
    in_maps = make_in_maps(x, W_qkv, b_qkv, W_o)
    res = run_bass_kernel_spmd(
        nc, in_maps, core_ids=list(range(NCORES)), trace=trace, **kw
    )
    parts = [np.asarray(m["y"], dtype=np.float32) for m in res.results]
    yb = []
    for b in range(B):
        yy = parts[4 * b]
        for g in range(1, 4):
            yy = yy + parts[4 * b + g]
        yb.append(yy)
    yout = np.stack(yb, axis=0)
    # exact bias correction: v-bias flows through attention (rows sum to 1)
    # into out-proj; k-bias cancels in softmax; q-bias applied on device.
    corr = W_o @ b_qkv[2 * D : 3 * D] + b_o
    yout = yout + corr[None, None, :]
    return yout, res


def kernel(x, W_qkv, b_qkv, W_o, b_o):
    y, _ = run({"x": x, "W_qkv": W_qkv, "b_qkv": b_qkv, "W_o": W_o, "b_o": b_o})
    return y
